# revision 11
# baseline (speedup 1.0000x reference)
"""Trainium2 Bass kernel for nn_CcLoss (gnn_message_passing).

Full inputs: features [64, 1024, 128] f32, tau scalar f32.
Data-parallel over batch B across 8 NeuronCores (8 samples per core).

Per sample b (on device):
  fn    = f / ||f||_rows                (row L2 norm over D)
  sim   = fn @ fn.T                     [P,P]  (PE, fp32r)
  mask  = sim > tau                     (0/1, bf16; DVE is_gt + ACT sigmoid-step)
  deg   = rowsum(mask)                  (fused accum_out of the compare ops)
  proto = (mask @ f) / deg              (PE bf16, mask is symmetric so its row
                                         tiles serve directly as lhsT)
  stats: Sum f^2, Sum proto*f, Sum proto^2 (per-partition accum + ones-matmul),
         gtsum[d] = Sum_p proto[p,d]    (ones-matmul)
Host combines stats into MSE + Pearson loss (exact algebra of the reference).
"""

import numpy as np

B, P, D = 64, 1024, 128
NCORES = 8
BLOC = B // NCORES          # samples per core
NT = P // 128               # 128-row tiles per sample
ROW = 160                   # per-sample stats row stride in the output
K_SIG = float(2 ** 40)      # sigmoid step sharpness (exact 0/1 in fp32)

# m-tiles whose mask compare runs on the scalar engine (sigmoid step);
# the rest run on the vector engine (is_gt). Tune for engine balance.
ACT_MTS = (0, 1, 2, 3, 4)

_PROG = None


def _build_program():
    import concourse.tile as tile
    from concourse import bacc, mybir, masks

    f32 = mybir.dt.float32
    bf16 = mybir.dt.bfloat16
    f32r = mybir.dt.float32r
    AF = mybir.ActivationFunctionType
    OP = mybir.AluOpType

    nc = bacc.Bacc(
        "TRN2",
        target_bir_lowering=False,
        debug=False,
        enable_asserts=False,
        num_devices=NCORES,
    )
    feats = nc.dram_tensor("features", [BLOC, P, D], f32, kind="ExternalInput").ap()
    tau_d = nc.dram_tensor("tau", [1, 1], f32, kind="ExternalInput").ap()
    out_d = nc.dram_tensor("out", [1, BLOC * ROW], f32, kind="ExternalOutput").ap()

    with tile.TileContext(nc) as tc:
        from contextlib import ExitStack

        with ExitStack() as ctx:
            const = ctx.enter_context(tc.tile_pool(name="const", bufs=1))
            fpool = ctx.enter_context(tc.tile_pool(name="f", bufs=2))
            fnpool = ctx.enter_context(tc.tile_pool(name="fn", bufs=2))
            b16pool = ctx.enter_context(tc.tile_pool(name="f16", bufs=2))
            ftpool = ctx.enter_context(tc.tile_pool(name="fnT", bufs=2))
            mpool = ctx.enter_context(tc.tile_pool(name="mask", bufs=2))
            ppool = ctx.enter_context(tc.tile_pool(name="proto", bufs=2))
            stpool = ctx.enter_context(tc.tile_pool(name="stat", bufs=2))
            smpool = ctx.enter_context(tc.tile_pool(name="small", bufs=4))
            dscr = ctx.enter_context(tc.tile_pool(name="dscr", bufs=2))
            gscr = ctx.enter_context(tc.tile_pool(name="gscr", bufs=2))
            pss_pool = ctx.enter_context(tc.tile_pool(name="pss", bufs=3, space="PSUM"))
            pmm_pool = ctx.enter_context(tc.tile_pool(name="pmm", bufs=3, space="PSUM"))
            pg_pool = ctx.enter_context(tc.tile_pool(name="pg", bufs=1, space="PSUM"))
            pst_pool = ctx.enter_context(tc.tile_pool(name="pstat", bufs=1, space="PSUM"))

            ident = const.tile([128, 128], f32)
            masks.make_identity(nc, ident[:])
            ones = const.tile([128, 1], f32)
            nc.gpsimd.memset(ones[:], 1.0)
            tau_bc = const.tile([128, 1], f32)
            nc.sync.dma_start(tau_bc[:], tau_d[0, :].partition_broadcast(128))
            nktau = const.tile([128, 1], f32)
            nc.gpsimd.tensor_scalar_mul(nktau[:], tau_bc[:], -K_SIG)
            srow = const.tile([1, BLOC * ROW], f32)
            nc.gpsimd.memset(srow[:], 0.0)

            for s in range(BLOC):
                # ---- load sample: 8 tiles of [128 rows, 128 feat] ----
                fb = fpool.tile([128, NT * 128], f32, tag="fb")
                nc.sync.dma_start(
                    fb[:].rearrange("p (t d) -> p t d", t=NT),
                    feats[s].rearrange("(t p) d -> p t d", p=128),
                )

                statv = stpool.tile([128, 24], f32, tag="statv")

                # ---- row norms^2 into statv[:, t] (also = per-row Sum f^2) ----
                for t in range(NT):
                    sc = dscr.tile([128, 128], f32, tag="dscr")
                    nc.vector.scalar_tensor_tensor(
                        sc[:],
                        fb[:, t * 128:(t + 1) * 128],
                        1.0,
                        fb[:, t * 128:(t + 1) * 128],
                        op0=OP.mult,
                        op1=OP.mult,
                        accum_out=statv[:, t:t + 1],
                    )
                sroot = smpool.tile([128, 8], f32, tag="sroot")
                nc.scalar.activation(sroot[:], statv[:, 0:8], AF.Sqrt)
                rinv = smpool.tile([128, 8], f32, tag="rinv")
                nc.vector.reciprocal(rinv[:], sroot[:])

                # ---- normalized rows + bf16 copy of f ----
                fn = fnpool.tile([128, NT * 128], f32, tag="fn")
                fb16 = b16pool.tile([128, NT * 128], bf16, tag="fb16")
                for t in range(NT):
                    nc.gpsimd.tensor_scalar_mul(
                        fn[:, t * 128:(t + 1) * 128],
                        fb[:, t * 128:(t + 1) * 128],
                        rinv[:, t:t + 1],
                    )
                    nc.gpsimd.tensor_copy(
                        fb16[:, t * 128:(t + 1) * 128],
                        fb[:, t * 128:(t + 1) * 128],
                    )

                # ---- transpose fn -> fnT [D on partitions, P free] ----
                fnT = ftpool.tile([128, P], f32r, tag="fnT")
                for t in range(NT):
                    pst = pmm_pool.tile([128, 128], f32, tag="mm128")
                    nc.tensor.matmul(
                        pst[:],
                        fn[:, t * 128:(t + 1) * 128],
                        ident[:],
                        is_transpose=True,
                    )
                    if t % 2 == 0:
                        nc.vector.tensor_copy(fnT[:, t * 128:(t + 1) * 128], pst[:])
                    else:
                        nc.scalar.copy(fnT[:, t * 128:(t + 1) * 128], pst[:])

                # ---- similarity + mask (+ fused deg accums) ----
                mask_t = mpool.tile([128, NT * P], bf16, tag="mask")
                dacc = smpool.tile([128, 16], f32, tag="dacc")
                for mt in range(NT):
                    for nb in range(2):
                        pss = pss_pool.tile([128, 512], f32, tag="pss")
                        nc.tensor.matmul(
                            pss[:],
                            fnT[:, mt * 128:(mt + 1) * 128],
                            fnT[:, nb * 512:(nb + 1) * 512],
                            start=True,
                            stop=True,
                        )
                        blk = mask_t[:, mt * P + nb * 512: mt * P + (nb + 1) * 512]
                        acc = dacc[:, nb * 8 + mt: nb * 8 + mt + 1]
                        if mt in ACT_MTS:
                            nc.scalar.activation(
                                blk,
                                pss[:],
                                AF.Sigmoid,
                                bias=nktau[:],
                                scale=K_SIG,
                                accum_out=acc,
                            )
                        else:
                            nc.vector.tensor_scalar(
                                blk,
                                pss[:],
                                tau_bc[:],
                                None,
                                op0=OP.is_gt,
                                op1=OP.add,
                                accum_out=acc,
                            )

                degv = smpool.tile([128, 8], f32, tag="degv")
                nc.vector.tensor_tensor(
                    degv[:], dacc[:, 0:8], dacc[:, 8:16], op=OP.add
                )
                rdeg = smpool.tile([128, 8], f32, tag="rdeg")
                nc.vector.reciprocal(rdeg[:], degv[:])

                # ---- proto = (mask @ f) / deg ; per-row stats ----
                proto = ppool.tile([128, NT * 128], f32, tag="proto")
                for mt in range(NT):
                    psp = pmm_pool.tile([128, 128], f32, tag="mm128")
                    for kc in range(NT):
                        nc.tensor.matmul(
                            psp[:],
                            mask_t[:, kc * P + mt * 128: kc * P + (mt + 1) * 128],
                            fb16[:, kc * 128:(kc + 1) * 128],
                            start=(kc == 0),
                            stop=(kc == NT - 1),
                        )
                    pr = proto[:, mt * 128:(mt + 1) * 128]
                    nc.scalar.activation(
                        pr, psp[:], AF.Copy, bias=0.0, scale=rdeg[:, mt:mt + 1]
                    )
                    g1 = gscr.tile([128, 128], f32, tag="gscr")
                    nc.vector.scalar_tensor_tensor(
                        g1[:],
                        pr,
                        1.0,
                        fb[:, mt * 128:(mt + 1) * 128],
                        op0=OP.mult,
                        op1=OP.mult,
                        accum_out=statv[:, 8 + mt:9 + mt],
                    )
                    g2 = gscr.tile([128, 128], f32, tag="gscr")
                    nc.vector.scalar_tensor_tensor(
                        g2[:],
                        pr,
                        1.0,
                        pr,
                        op0=OP.mult,
                        op1=OP.mult,
                        accum_out=statv[:, 16 + mt:17 + mt],
                    )

                # ---- gtsum[d] = Sum_p proto[p, d] ----
                psg = pg_pool.tile([128, 128], f32, tag="psg")
                for mt in range(NT):
                    nc.tensor.matmul(
                        psg[0:1, :],
                        ones[:, 0:1],
                        proto[:, mt * 128:(mt + 1) * 128],
                        start=(mt == 0),
                        stop=(mt == NT - 1),
                    )

                # ---- partition-sum the per-row stats ----
                pstat = pst_pool.tile([128, 24], f32, tag="pstat")
                nc.tensor.matmul(
                    pstat[0:1, :], ones[:, 0:1], statv[:], start=True, stop=True
                )
                nc.scalar.copy(srow[0:1, s * ROW: s * ROW + 24], pstat[0:1, :])
                nc.scalar.copy(srow[0:1, s * ROW + 32: s * ROW + 160], psg[0:1, :])

            nc.sync.dma_start(out_d[:], srow[:])

    nc.compile()
    return nc


def _get_program():
    global _PROG
    if _PROG is None:
        _PROG = _build_program()
    return _PROG


def _host_reduce(rows: np.ndarray) -> np.float32:
    """rows: [B, ROW] f32 per-sample device stats -> scalar loss."""
    rows = rows.astype(np.float64)
    N = float(P * D)
    ff = rows[:, 0:8].sum(axis=1)          # Sum f^2
    pf = rows[:, 8:16].sum(axis=1)         # Sum proto*f
    pp = rows[:, 16:24].sum(axis=1)        # Sum proto^2
    gtsum = rows[:, 32:160]                # Sum_p proto  [B, D]

    mse = (pp - 2.0 * pf + ff) / N
    sum_proto = gtsum.sum(axis=1)
    gtm = gtsum / float(P)
    ybar = sum_proto / N
    S = ((gtm - ybar[:, None]) ** 2).sum(axis=1)
    sum_xc2 = pp - (sum_proto ** 2) / N
    num = float(P) * S
    corr = num / np.sqrt(sum_xc2 * num)
    loss = mse.mean() + (0.5 * (corr + 1.0)).mean()
    return np.float32(loss)


_LAST_RESULTS = None


def kernel(features: np.ndarray, tau: np.ndarray, **run_kwargs) -> np.ndarray:
    global _LAST_RESULTS
    from concourse import bass_utils

    features = np.ascontiguousarray(features, dtype=np.float32)
    tau_v = np.array(tau, dtype=np.float32).reshape(1, 1)

    nc = _get_program()
    shards = features.reshape(NCORES, BLOC, P, D)
    in_maps = [
        {"features": shards[i], "tau": tau_v.copy()} for i in range(NCORES)
    ]
    res = bass_utils.run_bass_kernel_spmd(
        nc, in_maps, core_ids=list(range(NCORES)), **run_kwargs
    )
    _LAST_RESULTS = res
    rows = np.concatenate(
        [res.results[i]["out"].reshape(BLOC, ROW) for i in range(NCORES)], axis=0
    )
    return _host_reduce(rows)


if __name__ == "__main__":
    x = np.random.randn(B, P, D).astype(np.float32)
    t = np.float32(0.5)
    print(kernel(x, t))


# revision 13
# speedup vs baseline: 1.3109x; 1.3109x over previous
"""Trainium2 Bass kernel for nn_CcLoss (gnn_message_passing).

Full inputs: features [64, 1024, 128] f32, tau scalar f32.
Data-parallel over batch B across 8 NeuronCores (8 samples per core).

Per sample b (on device):
  fn    = f / ||f||_rows                (row L2 norm over D)
  sim   = fn @ fn.T                     [P,P]  (PE, fp32r)
  mask  = sim > tau                     (0/1, bf16; ACT sigmoid-step + DVE is_gt,
                                         one [128,1024] op per row-tile)
  deg   = rowsum(mask)                  (fused accum_out of the compare ops)
  proto = (mask @ f) / deg              (PE bf16, mask is symmetric so its row
                                         tiles serve directly as lhsT)
  stats: Sum f^2 (per row), Sum proto*f, Sum proto^2, gtsum[d] = Sum_p proto
Host combines stats into MSE + Pearson loss (exact algebra of the reference).
"""

import numpy as np

B, P, D = 64, 1024, 128
NCORES = 8
BLOC = B // NCORES          # samples per core
NT = P // 128               # 128-row tiles per sample
ROW = 160                   # per-sample stats row stride in the output
K_SIG = float(2 ** 40)      # sigmoid step sharpness (exact 0/1 in fp32)

_PROG = None


def _act_mt_count(s: int) -> int:
    # compare row-tiles handled by ScalarE for sample s (rest on VectorE)
    return 6 if s % 2 == 0 else 5


def _build_program():
    import concourse.tile as tile
    from concourse import bacc, mybir, masks

    f32 = mybir.dt.float32
    bf16 = mybir.dt.bfloat16
    f32r = mybir.dt.float32r
    AF = mybir.ActivationFunctionType
    OP = mybir.AluOpType

    nc = bacc.Bacc(
        "TRN2",
        target_bir_lowering=False,
        debug=False,
        enable_asserts=False,
        num_devices=NCORES,
    )
    feats = nc.dram_tensor("features", [BLOC, P, D], f32, kind="ExternalInput").ap()
    tau_d = nc.dram_tensor("tau", [1, 1], f32, kind="ExternalInput").ap()
    out_d = nc.dram_tensor("out", [1, BLOC * ROW], f32, kind="ExternalOutput").ap()

    with tile.TileContext(nc) as tc:
        from contextlib import ExitStack

        with ExitStack() as ctx:
            const = ctx.enter_context(tc.tile_pool(name="const", bufs=1))
            fpool = ctx.enter_context(tc.tile_pool(name="f", bufs=2))
            fnpool = ctx.enter_context(tc.tile_pool(name="fn", bufs=2))
            b16pool = ctx.enter_context(tc.tile_pool(name="f16", bufs=2))
            ftpool = ctx.enter_context(tc.tile_pool(name="fnT", bufs=2))
            mpool = ctx.enter_context(tc.tile_pool(name="mask", bufs=2))
            ppool = ctx.enter_context(tc.tile_pool(name="proto", bufs=2))
            stpool = ctx.enter_context(tc.tile_pool(name="stat", bufs=3))
            smpool = ctx.enter_context(tc.tile_pool(name="small", bufs=4))
            dscr = ctx.enter_context(tc.tile_pool(name="dscr", bufs=2))
            gscr = ctx.enter_context(tc.tile_pool(name="gscr", bufs=2))
            pss_pool = ctx.enter_context(tc.tile_pool(name="pss", bufs=2, space="PSUM"))
            pmm_pool = ctx.enter_context(tc.tile_pool(name="pmm", bufs=2, space="PSUM"))
            pg_pool = ctx.enter_context(tc.tile_pool(name="pg", bufs=1, space="PSUM"))
            pst_pool = ctx.enter_context(tc.tile_pool(name="pstat", bufs=1, space="PSUM"))

            ident = const.tile([128, 128], f32)
            masks.make_identity(nc, ident[:])
            ones = const.tile([128, 1], f32)
            nc.gpsimd.memset(ones[:], 1.0)
            tau_bc = const.tile([128, 1], f32)
            nc.sync.dma_start(tau_bc[:], tau_d[0, :].partition_broadcast(128))
            nktau = const.tile([128, 1], f32)
            nc.gpsimd.tensor_scalar_mul(nktau[:], tau_bc[:], -K_SIG)
            srow = const.tile([1, BLOC * ROW], f32)
            nc.gpsimd.memset(srow[:], 0.0)

            for s in range(BLOC):
                # ---- load sample as 8 [128,128] tiles packed in [128, 1024] ----
                fb = fpool.tile([128, NT * 128], f32, tag="fb")
                nc.sync.dma_start(
                    fb[:].rearrange("p (t d) -> p t d", t=NT),
                    feats[s].rearrange("(t p) d -> p t d", p=128),
                )

                statv = stpool.tile([128, 10], f32, tag="statv")

                # ---- row norms^2 -> statv[:, t] : ACT square + DVE reduce ----
                sq = dscr.tile([128, NT * 128], f32, tag="dscr")
                nc.scalar.activation(sq[:], fb[:], AF.Square)
                nc.vector.tensor_reduce(
                    statv[:, 0:8],
                    sq[:].rearrange("p (t d) -> p t d", t=NT),
                    axis=mybir.AxisListType.X,
                    op=OP.add,
                )
                sroot = smpool.tile([128, 8], f32, tag="sroot")
                nc.scalar.activation(sroot[:], statv[:, 0:8], AF.Sqrt)
                rinv = smpool.tile([128, 8], f32, tag="rinv")
                nc.vector.reciprocal(rinv[:], sroot[:])

                # ---- normalized rows (DVE) + bf16 copy of f (DVE, one op) ----
                fn = fnpool.tile([128, NT * 128], f32, tag="fn")
                for t in range(NT):
                    nc.vector.tensor_scalar_mul(
                        fn[:, t * 128:(t + 1) * 128],
                        fb[:, t * 128:(t + 1) * 128],
                        rinv[:, t:t + 1],
                    )
                fb16 = b16pool.tile([128, NT * 128], bf16, tag="fb16")
                nc.vector.tensor_copy(fb16[:], fb[:])

                # ---- transpose fn -> fnT [D, P]; 4 transposes per PSUM bank ----
                fnT = ftpool.tile([128, P], f32r, tag="fnT")
                for h in range(2):
                    pst = pmm_pool.tile([128, 512], f32, tag="mm512")
                    for q in range(4):
                        t = h * 4 + q
                        nc.tensor.matmul(
                            pst[:, q * 128:(q + 1) * 128],
                            fn[:, t * 128:(t + 1) * 128],
                            ident[:],
                            is_transpose=True,
                        )
                    if h == 0:
                        nc.vector.tensor_copy(fnT[:, h * 512:(h + 1) * 512], pst[:])
                    else:
                        nc.scalar.copy(fnT[:, h * 512:(h + 1) * 512], pst[:])

                # ---- similarity + mask (+ fused deg accum), one op per mt ----
                mask_t = mpool.tile([128, NT * P], bf16, tag="mask")
                dacc = smpool.tile([128, 8], f32, tag="dacc")
                n_act = _act_mt_count(s)
                for mt in range(NT):
                    pss = pss_pool.tile([128, 1024], f32, tag="pss")
                    for nb in range(2):
                        nc.tensor.matmul(
                            pss[:, nb * 512:(nb + 1) * 512],
                            fnT[:, mt * 128:(mt + 1) * 128],
                            fnT[:, nb * 512:(nb + 1) * 512],
                            start=True,
                            stop=True,
                        )
                    blk = mask_t[:, mt * P:(mt + 1) * P]
                    acc = dacc[:, mt:mt + 1]
                    if mt < n_act:
                        nc.scalar.activation(
                            blk, pss[:], AF.Sigmoid,
                            bias=nktau[:], scale=K_SIG, accum_out=acc,
                        )
                    else:
                        nc.vector.tensor_scalar(
                            blk, pss[:], tau_bc[:], None,
                            op0=OP.is_gt, op1=OP.add, accum_out=acc,
                        )

                rdeg = smpool.tile([128, 8], f32, tag="rdeg")
                nc.vector.reciprocal(rdeg[:], dacc[:])

                # ---- proto = (mask @ f) / deg ; 4 m-tiles per PSUM bank ----
                proto = ppool.tile([128, NT * 128], f32, tag="proto")
                for h in range(2):
                    psp = pmm_pool.tile([128, 512], f32, tag="mm512")
                    for q in range(4):
                        mt = h * 4 + q
                        for kc in range(NT):
                            nc.tensor.matmul(
                                psp[:, q * 128:(q + 1) * 128],
                                mask_t[:, kc * P + mt * 128: kc * P + (mt + 1) * 128],
                                fb16[:, kc * 128:(kc + 1) * 128],
                                start=(kc == 0),
                                stop=(kc == NT - 1),
                            )
                    # normalize: proto[:, h half] = psp * rdeg (per-row scalar)
                    for q in range(4):
                        mt = h * 4 + q
                        nc.scalar.activation(
                            proto[:, mt * 128:(mt + 1) * 128],
                            psp[:, q * 128:(q + 1) * 128],
                            AF.Copy, bias=0.0, scale=rdeg[:, mt:mt + 1],
                        )

                # ---- Sum proto*f and Sum proto^2, one DVE op each ----
                g1 = gscr.tile([128, NT * 128], f32, tag="gscr")
                nc.vector.scalar_tensor_tensor(
                    g1[:], proto[:], 1.0, fb[:],
                    op0=OP.mult, op1=OP.mult, accum_out=statv[:, 8:9],
                )
                g2 = gscr.tile([128, NT * 128], f32, tag="gscr")
                nc.vector.scalar_tensor_tensor(
                    g2[:], proto[:], 1.0, proto[:],
                    op0=OP.mult, op1=OP.mult, accum_out=statv[:, 9:10],
                )

                # ---- gtsum[d] = Sum_p proto[p, d] ----
                psg = pg_pool.tile([128, 128], f32, tag="psg")
                for mt in range(NT):
                    nc.tensor.matmul(
                        psg[0:1, :],
                        ones[:, 0:1],
                        proto[:, mt * 128:(mt + 1) * 128],
                        start=(mt == 0),
                        stop=(mt == NT - 1),
                    )

                # ---- partition-sum the per-row stats ----
                pstat = pst_pool.tile([128, 10], f32, tag="pstat")
                nc.tensor.matmul(
                    pstat[0:1, :], ones[:, 0:1], statv[:], start=True, stop=True
                )
                nc.scalar.copy(srow[0:1, s * ROW: s * ROW + 10], pstat[0:1, :])
                nc.scalar.copy(srow[0:1, s * ROW + 32: s * ROW + 160], psg[0:1, :])

            nc.sync.dma_start(out_d[:], srow[:])

    nc.compile()
    return nc


def _get_program():
    global _PROG
    if _PROG is None:
        _PROG = _build_program()
    return _PROG


def _host_reduce(rows: np.ndarray) -> np.float32:
    """rows: [B, ROW] f32 per-sample device stats -> scalar loss."""
    rows = rows.astype(np.float64)
    N = float(P * D)
    ff = rows[:, 0:8].sum(axis=1)          # Sum f^2
    pf = rows[:, 8]                        # Sum proto*f
    pp = rows[:, 9]                        # Sum proto^2
    gtsum = rows[:, 32:160]                # Sum_p proto  [B, D]

    mse = (pp - 2.0 * pf + ff) / N
    sum_proto = gtsum.sum(axis=1)
    gtm = gtsum / float(P)
    ybar = sum_proto / N
    S = ((gtm - ybar[:, None]) ** 2).sum(axis=1)
    sum_xc2 = pp - (sum_proto ** 2) / N
    num = float(P) * S
    corr = num / np.sqrt(sum_xc2 * num)
    loss = mse.mean() + (0.5 * (corr + 1.0)).mean()
    return np.float32(loss)


_LAST_RESULTS = None


def kernel(features: np.ndarray, tau: np.ndarray, **run_kwargs) -> np.ndarray:
    global _LAST_RESULTS
    from concourse import bass_utils

    features = np.ascontiguousarray(features, dtype=np.float32)
    tau_v = np.array(tau, dtype=np.float32).reshape(1, 1)

    nc = _get_program()
    shards = features.reshape(NCORES, BLOC, P, D)
    in_maps = [
        {"features": shards[i], "tau": tau_v.copy()} for i in range(NCORES)
    ]
    res = bass_utils.run_bass_kernel_spmd(
        nc, in_maps, core_ids=list(range(NCORES)), **run_kwargs
    )
    _LAST_RESULTS = res
    rows = np.concatenate(
        [res.results[i]["out"].reshape(BLOC, ROW) for i in range(NCORES)], axis=0
    )
    return _host_reduce(rows)


if __name__ == "__main__":
    x = np.random.randn(B, P, D).astype(np.float32)
    t = np.float32(0.5)
    print(kernel(x, t))
